# revision 42
# baseline (speedup 1.0000x reference)
"""Causal single-head attention (B=8, T=2048, C=1024, H=128) on 8 TRN2 NeuronCores.

Sharding: data-parallel over batch B - one batch element per core; weights
replicated. Inputs cast to fp16 on host; x pre-transposed and chunk-packed,
weights packed chunk-major so every DMA descriptor is a contiguous run
>= 512B (full DMA rate).

Single fused stream per core. For each query block b (512 queries):
  s^T chunk = k_chunk @ q^T -> exp on ACT (scale=C^-0.5) -> causal tri-mask
  on the diagonal 128x128 block via Pool-engine multiply -> o[q,0:129]
  accumulated in natural layout with the p^T-block as the matmul STATIONARY
  operand and [v | ones] as the moving operand: column 128 accumulates the
  softmax denominator for free (no separate l matmul, no output transposes).
  o-accumulation is batched per 128-query block j with strictly sequential
  accumulation-group lifetimes per PSUM 2KB zero region (a later group's
  start re-arms zero-on-write for the whole region; reads stay safe).
  Early partial batches cover the off-diagonal prefix; final batches after
  the diagonal exps are only 1-4 matmuls. Epilogue per j: reciprocal of
  col 128 (DVE), scale into fp32 SBUF (DVE), DMA out (paired row-block
  DMAs for the last query block to halve tail DMA issue overhead).

Projections of x-block tb+1 weave into attention block b=tb as PE filler so
the PE never waits on ACT exp; dummy warm-up matmuls cover the initial x/W
DMA trickle and keep the PE p-state ramp early.
"""
import numpy as np

import concourse.bass as bass
import concourse.mybir as mybir
import concourse.tile as tile
from concourse import bacc
from concourse.bass_utils import run_bass_kernel_spmd

P = 128
T = 2048
C = 1024
H = 128
HP1 = H + 1          # v columns + ones column
CO = C // P          # 8 contraction chunks
TB = 512             # query block
NTB = T // TB        # 4
NKC = T // P         # 16 key chunks
F32 = mybir.dt.float32
F16 = mybir.dt.float16
SCALE = C ** -0.5    # 1/32, matches reference (scales by n_embed, not head_size)

N_CORES = 8


def build_nc(n_warm=23, s_bufs=4, ptile_bufs=20,
             warm_sprinkle=(2, 10, 0, 9, 0, 0, 0, 0), v_sprinkle=0,
             masks_on_pool=False, split_finals=True, paired_out=False,
             merge=4, seg3_wr=12, pair_exps=False):
    if pair_exps:
        s_bufs = min(s_bufs, 2)  # pair tiles are 2 banks each
        ptile_bufs = min(ptile_bufs, 10)
    nc = bacc.Bacc("TRN2", target_bir_lowering=False, debug=False,
                   enable_asserts=False, num_devices=N_CORES)
    xw = nc.dram_tensor("xw", [P, CO, T], F16, kind="ExternalInput")
    wall = nc.dram_tensor("wall", [P, CO, 3, H], F16, kind="ExternalInput")
    out = nc.dram_tensor("out", [T, H], F32, kind="ExternalOutput")
    out3 = out.rearrange("(n p) h -> p n h", p=P)   # [128, 16, 128]

    with tile.TileContext(nc) as tc:
        with (
            tc.tile_pool(name="const", bufs=1) as const,
            tc.tile_pool(name="persist", bufs=1) as persist,
            tc.tile_pool(name="xload", bufs=4) as xload,
            tc.tile_pool(name="ptile", bufs=ptile_bufs) as ptile,
            tc.tile_pool(name="osb", bufs=2) as osbp,
            tc.tile_pool(name="lin", bufs=8) as linp,
            tc.tile_pool(name="ps_s", bufs=s_bufs, space="PSUM") as ps_s,
            tc.tile_pool(name="ps_qk", bufs=1, space="PSUM") as ps_qk,
            tc.tile_pool(name="ps_v", bufs=1, space="PSUM") as ps_vp,
            tc.tile_pool(name="ps_oacc", bufs=2, space="PSUM") as ps_oacc,
        ):
            # ---- constants / warmup (warm memset first on the Pool stream) ----
            warm = const.tile([P, P], F16)
            nc.gpsimd.memset(warm[:], 0.0)
            tri = const.tile([P, P], F16)        # [k, q]: 1 where q >= k else 0
            nc.gpsimd.memset(tri[:], 1.0)
            nc.gpsimd.affine_select(
                out=tri[:], in_=tri[:],
                compare_op=mybir.AluOpType.is_ge, fill=0.0, base=0,
                pattern=[[1, P]], channel_multiplier=-1)
            weout = const.tile([P, 1], F16)
            # tiny exp first so the ACT function table loads during DMA startup
            nc.scalar.activation(weout[:1, :], warm[:1, 0:1],
                                 mybir.ActivationFunctionType.Exp)

            # ---- persistent activations ----
            q_T = persist.tile([P, T], F16)          # [H, T]
            k_T = persist.tile([P, T], F16)          # [H, T]
            v1 = persist.tile([P, NKC, HP1], F16)    # [t%128, kc, H+1]
            nc.gpsimd.memset(v1[:, :, H:HP1], 1.0)   # ones column

            wall_sb = const.tile([P, CO, 3, H], F16)

            # ---- DMAs (SP queue, priority order; x fully prefetched) ----
            xt = {tb: xload.tile([P, CO, TB], F16, name="xt") for tb in range(NTB)}
            x4 = xw.rearrange("p o (n t) -> p o n t", t=TB)  # [128, 8, 4, 512]
            nc.sync.dma_start(wall_sb[:, 0:1], wall[:, 0:1])
            nc.sync.dma_start(xt[0][:, 0:1, :], x4[:, 0:1, 0, :])
            nc.sync.dma_start(xt[0][:, 1:2, :], x4[:, 1:2, 0, :])
            nc.sync.dma_start(wall_sb[:, 1:4], wall[:, 1:4])
            nc.sync.dma_start(xt[0][:, 2:3, :], x4[:, 2:3, 0, :])
            nc.sync.dma_start(xt[0][:, 3:4, :], x4[:, 3:4, 0, :])
            nc.sync.dma_start(wall_sb[:, 4:8], wall[:, 4:8])
            for c in range(4, 8):
                nc.sync.dma_start(xt[0][:, c:c + 1, :], x4[:, c:c + 1, 0, :])
            for tb in range(1, NTB):
                nc.sync.dma_start(xt[tb][:, 0:4, :], x4[:, 0:4, tb, :])
                nc.sync.dma_start(xt[tb][:, 4:8, :], x4[:, 4:8, tb, :])

            # ---- PE warm-up matmuls on zero data (p-state ramp) ----
            ps_warm = ps_vp.tile([P, 2, 256], F32, name="ps_v")

            def warm_item():
                nc.tensor.matmul(ps_warm[:, 0, 0:P], warm[:], warm[:],
                                 start=True, stop=True)

            for _ in range(n_warm):
                warm_item()

            # ================= emission helpers =================
            proj_state = {}

            def copy(eng, dst, src):
                if eng == "act":
                    nc.scalar.activation(dst, src,
                                         mybir.ActivationFunctionType.Copy)
                else:
                    nc.vector.tensor_copy(dst, src)

            def emit_proj(tb, which, c, copy_eng="dve"):
                """One contraction-chunk matmul of q^T (which=0) or k^T (1).

                q uses the dedicated ps_qk bank; k borrows a ps_s buffer so
                q/k chunk matmuls can interleave during the DMA-paced
                prologue without a second dedicated bank.
                """
                dtile = q_T if which == 0 else k_T
                if c == 0:
                    if which == 0:
                        proj_state[0] = ps_qk.tile([P, TB], F32,
                                                   name="ps_qk")[:, :]
                    elif pair_exps:
                        proj_state[1] = ps_s.tile([P, 2, TB], F32,
                                                  name="s_ps")[:, 0, :]
                    else:
                        proj_state[1] = ps_s.tile([P, TB], F32,
                                                  name="s_ps")[:, :]
                st = proj_state[which]
                nc.tensor.matmul(st, wall_sb[:, c, which, :], xt[tb][:, c, :],
                                 start=(c == 0), stop=(c == CO - 1))
                if c == CO - 1:
                    tsl = slice(tb * TB, (tb + 1) * TB)
                    copy(copy_eng, dtile[:, tsl], st)

            def emit_proj_v(tb, j, copy_eng="dve"):
                """All 8 contraction chunks for one 128-row block of v."""
                jj = j % 2
                if jj == 0:
                    proj_state["v"] = ps_vp.tile([P, 2, 256], F32, name="ps_v")
                st = proj_state["v"]
                for c in range(CO):
                    nc.tensor.matmul(st[:, jj, 0:H],
                                     xt[tb][:, c, j * P:(j + 1) * P],
                                     wall_sb[:, c, 2, :],
                                     start=(c == 0), stop=(c == CO - 1))
                if jj == 1:
                    n0 = tb * 4 + j - 1
                    copy(copy_eng, v1[:, n0:n0 + 2, 0:H], st[:, :, 0:H])

            def proj_items(tb, parts="qkv"):
                items = []
                if "q" in parts:
                    items += [(emit_proj, (tb, 0, c)) for c in range(CO)]
                if "k" in parts:
                    items += [(emit_proj, (tb, 1, c)) for c in range(CO)]
                if "v" in parts:
                    items += [(emit_proj_v, (tb, j)) for j in range(4)]
                return items

            def weave(items, filler, wr=None):
                nf = len(filler)
                wr = wr if wr is not None else len(items)
                done = 0
                for idx, (fn, args) in enumerate(items):
                    fn(*args)
                    want = min(nf, int(round(nf * (idx + 1) / max(1, wr))))
                    while done < want:
                        ffn, fargs = filler[done]
                        ffn(*fargs)
                        done += 1
                while done < nf:
                    ffn, fargs = filler[done]
                    ffn(*fargs)
                    done += 1

            class Block:
                """Attention emission for one 512-query block b."""

                def __init__(self, b):
                    self.b = b
                    self.pts = {}
                    self.oacc = None
                    self.osb = None

                def _init_tiles(self):
                    if self.oacc is None:
                        # j0,j2 share tile0; j1,j3 share tile1 - group
                        # lifetimes per 2KB region stay disjoint
                        self.oacc = [ps_oacc.tile([P, 2, 256], F32, name="oacc")
                                     for _ in range(2)]
                        self.osb = osbp.tile([P, 4, H], F32, name="osb")

                def s_item(self, i):
                    """Key-chunk i; even off-diagonal chunks open a pair tile
                    and defer the exp so chunks i, i+1 share ONE 1024-col ACT
                    exp (halves per-instruction ACT init overhead)."""
                    b = self.b
                    d = i - 4 * b
                    off = max(d, 0) * P
                    w = TB - off
                    if pair_exps and d < 0:
                        if i % 2 == 0:
                            self.sp = ps_s.tile([P, 2, TB], F32, name="s_ps")
                            self.pp = ptile.tile([P, 2, TB], F16, name="pTp")
                        ps2, pT2 = self.sp, self.pp
                        nc.tensor.matmul(ps2[:, i % 2, :],
                                         k_T[:, i * P:(i + 1) * P],
                                         q_T[:, b * TB:(b + 1) * TB],
                                         start=True, stop=True)
                        if i % 2 == 1:  # one exp for both chunks
                            nc.scalar.activation(
                                pT2[:], ps2[:],
                                mybir.ActivationFunctionType.Exp, scale=SCALE)
                        self.pts[i] = pT2[:, i % 2, :]
                        return
                    ps = ps_s.tile([P, 2, TB], F32, name="s_ps")[:, 0, :] \
                        if pair_exps else ps_s.tile([P, TB], F32, name="s_ps")
                    nc.tensor.matmul(ps[0:P, 0:w], k_T[:, i * P:(i + 1) * P],
                                     q_T[:, b * TB + off:(b + 1) * TB],
                                     start=True, stop=True)
                    pT = ptile.tile([P, TB], F16, name="pT")
                    nc.scalar.activation(pT[:, off:TB], ps[0:P, 0:w],
                                         mybir.ActivationFunctionType.Exp,
                                         scale=SCALE)
                    if d >= 0:  # causal mask on the diagonal block
                        if masks_on_pool:
                            nc.gpsimd.tensor_tensor(pT[:, off:off + P],
                                                    pT[:, off:off + P], tri[:],
                                                    mybir.AluOpType.mult)
                        else:
                            nc.vector.tensor_tensor(pT[:, off:off + P],
                                                    pT[:, off:off + P], tri[:],
                                                    mybir.AluOpType.mult)
                    self.pts[i] = pT

                def o_part(self, j, lo, hi, is_stop):
                    self._init_tiles()
                    b, t = self.b, self.oacc[j % 2]
                    jj = j // 2
                    for i in range(lo, hi + 1):
                        nc.tensor.matmul(t[:, jj, 0:HP1],
                                         self.pts[i][:, j * P:(j + 1) * P],
                                         v1[:, i, :],
                                         start=(i == 0),
                                         stop=(is_stop and i == hi))
                    if not is_stop:
                        return
                    lin = linp.tile([P, 1], F32, name="lin")
                    nc.vector.reciprocal(lin[:], t[:, jj, H:HP1])
                    nc.vector.tensor_tensor(
                        self.osb[:, j:j + 1, :], t[:, jj:jj + 1, 0:H],
                        lin[:, :, None].to_broadcast([P, 1, H]),
                        mybir.AluOpType.mult)
                    if b < 3:
                        if j == 3:
                            nc.sync.dma_start(out3[:, b * 4:(b + 1) * 4, :],
                                              self.osb[:])
                    elif paired_out:
                        if j in (1, 3):
                            nc.sync.dma_start(out3[:, 12 + j - 1:13 + j, :],
                                              self.osb[:, j - 1:j + 1, :])
                    else:
                        nc.sync.dma_start(out3[:, 12 + j:13 + j, :],
                                          self.osb[:, j:j + 1, :])

                def body_items(self):
                    return [(self.s_item, (i,)) for i in range(4 * self.b)]

                def diag_items(self):
                    b, d0 = self.b, 4 * self.b
                    pe = d0 - 1 if split_finals else d0 + 1
                    items = []
                    if b > 0:
                        items += [(self.o_part, (0, 0, d0 - 1, False)),
                                  (self.o_part, (1, 0, d0 - 1, False))]
                    items += [
                        (self.s_item, (d0,)),
                        (self.o_part, (0, d0, d0, True)),
                        (self.s_item, (d0 + 1,)),
                        (self.o_part, (1, d0, d0 + 1, True)),
                    ]
                    if b > 0:
                        items += [(self.o_part, (2, 0, pe, False)),
                                  (self.o_part, (3, 0, pe, False))]
                    items += [
                        (self.s_item, (d0 + 2,)),
                        (self.o_part, (2, (pe + 1) if b > 0 else 0, d0 + 2, True)),
                        (self.s_item, (d0 + 3,)),
                        (self.o_part, (3, (pe + 1) if b > 0 else 0, d0 + 3, True)),
                    ]
                    return items

            # ================= schedule =================
            # prologue: tb0 projections paced against the x/W DMA trickle,
            # warm matmuls sprinkled into the DMA-bound stretch. q/k copies
            # split across DVE/ACT so the b0 boundary isn't copy-serial.
            for c in range(CO):
                emit_proj(0, 0, c, copy_eng="dve")
                emit_proj(0, 1, c, copy_eng="act")
                for _ in range(warm_sprinkle[c]):
                    warm_item()
            for j in range(4):
                emit_proj_v(0, j, copy_eng="act" if j >= 2 else "dve")
                if j == 1:
                    for _ in range(v_sprinkle):
                        warm_item()

            b0, b1, b2, b3 = Block(0), Block(1), Block(2), Block(3)
            b3_body = b3.body_items()
            weave(b0.body_items() + b0.diag_items(), proj_items(1))
            weave(b1.body_items() + b1.diag_items(), proj_items(2))
            # optionally pull b3's first body s-items into seg2 so their exps
            # drain during seg2's PE-heavy window
            weave(b2.body_items() + b2.diag_items(),
                  proj_items(3, "q") + b3_body[:merge])
            weave(b3_body[merge:] + b3.diag_items(),
                  proj_items(3, "k") + proj_items(3, "v"), wr=seg3_wr)

    nc.compile()
    return nc


_NC = None


def _get_nc():
    global _NC
    if _NC is None:
        _NC = build_nc()
    return _NC


def kernel(x, Wq, Wk, Wv):
    x = np.asarray(x)
    B = x.shape[0]
    assert B == N_CORES and x.shape[1:] == (T, C)
    # x^T chunk-packed: [B, 128, 8, T] with c = o*128 + p
    x16 = np.ascontiguousarray(
        x.astype(np.float16).transpose(0, 2, 1).reshape(B, CO, P, T)
        .transpose(0, 2, 1, 3))
    # weights packed chunk-major to SBUF layout [128, 8, 3, 128]
    wall = np.stack([np.asarray(Wq), np.asarray(Wk), np.asarray(Wv)], axis=0)
    wall = np.ascontiguousarray(
        wall.astype(np.float16).reshape(3, CO, P, H).transpose(2, 1, 0, 3))

    nc = _get_nc()
    in_maps = [{"xw": x16[b], "wall": wall} for b in range(B)]
    res = run_bass_kernel_spmd(nc, in_maps, core_ids=list(range(N_CORES)))
    return np.stack([r["out"] for r in res.results], axis=0)


if __name__ == "__main__":
    rng = np.random.default_rng(0)
    x = rng.standard_normal((8, T, C), dtype=np.float32)
    s = C ** -0.5
    Wq = rng.standard_normal((C, H), dtype=np.float32) * s
    Wk = rng.standard_normal((C, H), dtype=np.float32) * s
    Wv = rng.standard_normal((C, H), dtype=np.float32) * s
    out = kernel(x, Wq, Wk, Wv)
    print(out.shape, out.dtype)
